# revision 4
# baseline (speedup 1.0000x reference)
"""Global-average-pool + sigmoid channel scores on 8 trn2 NeuronCores — v4.

v3 (150.1us): no collectives, host-side finish, 8x784 tail split, split
out-DMA.  The v3 trace shows HBM streaming can burst to ~396 GB/s, at which
point the DVE reduce (1.13 ns/elem ~= 113 Gelem/s vs the stream's 99
Gelem/s) plus the 6-buffer window become co-limiting: 3.6us + 2.6us ring
stalls appeared near the end of the stream.

v4: alternate piece reduces between DVE (vector.reduce_sum) and the Scalar
(ACT) engine, whose activation(func=Copy, accum_out=...) computes the same
per-partition free-axis sum.  Combined reduce throughput ~2x; neither engine
can stall the DMA ring.  bufs 6->7 widens the DMA run-ahead window.  ACT's
mandatory elementwise output goes to a scratch tile nothing reads.

v5 (v4 measured 137.5us, stream at 421 GB/s): v4's early out-DMA sat in the
Sync queue waiting on all 15 full-piece reduces, blocking the tail data
DMAs queued behind it (~4.8us of ring gaps).  Issue it from GpSimd (SWDGE)
instead — GpSimd is otherwise idle, so the wait blocks nothing.

v6 (v5 measured 163.9us at 342 GB/s — HBM bandwidth varies 342-421 GB/s run
to run with neighbor-core skew; structurally v5's stream was gap-free with a
2.9us tail).  Micro-polish: ACT (0.68 ns/elem) is faster than DVE (1.13
ns/elem), so give ACT the even pieces incl. the final one; shrink the last
two tail pieces to 392 cols; split the first chunk 1568+4704 so the first
descgen is short and the stream ramps sooner.

v7 (v6 measured 149.6us at 383 GB/s, but tail grew to 5.1us): when the
stream is fast, the final full-width reduce (7.1us on DVE) spills past
stream-end, serializing the tail reduces behind it and delaying a tail DMA
through buffer recycling.  Taper the last TWO chunks geometrically
(3136,3136,1568,1568,1176,784,588,392,196) so every reduce near the end
finishes inside the remaining stream runway even at 435 GB/s; worst-case
tail is then ~receipt(1us) + 0.25us reduce + out-DMA ~= 2.7us.
"""

import numpy as np

try:
    import concourse.bass as bass  # noqa: F401
except ImportError:  # pragma: no cover - fallback when site path is absent
    import sys

    for p in ("/opt/trn_rl_repo", "/root/.axon_site/_ro/trn_rl_repo"):
        if p not in sys.path:
            sys.path.insert(0, p)

import concourse.bass as bass
import concourse.bacc as bacc
import concourse.mybir as mybir
import concourse.tile as tile
from concourse.bass_utils import run_bass_kernel_spmd

N_CORES = 8
B, C, H, W = 32, 64, 224, 224
B_LOC = B // N_CORES            # 4 batches per core
ROWS = B_LOC * C                # 256 (b_loc, c) rows per core
HW = H * W                      # 50176 spatial elements per row
N_PTILES = ROWS // 128          # 2 partition tiles of 128 rows
CHUNK = 6272                    # 50176 = 8 * 6272; 3.2 MB per DMA tile
N_CHUNKS = HW // CHUNK          # 8 free-dim chunks per partition tile
MEAN_SCALE = 1.0 / (B * HW)     # mean over batch+spatial = 32*50176 elems
HEAD_SPLIT = [1568, 4704]       # first chunk: short first descgen
# geometric taper over the last TWO chunks (12544 cols): no reduce near the
# end is long enough to spill past stream-end even at 435 GB/s
TAIL_SPLIT = [3136, 3136, 1568, 1568, 1176, 784, 588, 392, 196]
N_TAIL_CHUNKS = 2
assert sum(TAIL_SPLIT) == N_TAIL_CHUNKS * CHUNK
N_FULL = (N_PTILES * N_CHUNKS - N_TAIL_CHUNKS - 1) + len(HEAD_SPLIT)
N_PIECES = N_FULL + len(TAIL_SPLIT)
DATA_BUFS = 7

_CACHE = {}


def _pieces():
    """(row_tile, col, width)"""
    pieces = []
    for n in range(N_PTILES):
        for j in range(N_CHUNKS):
            base = j * CHUNK
            if n == 0 and j == 0:
                col = base
                for w in HEAD_SPLIT:
                    pieces.append((n, col, w))
                    col += w
            elif n == N_PTILES - 1 and j == N_CHUNKS - N_TAIL_CHUNKS:
                col = base
                for w in TAIL_SPLIT:
                    pieces.append((n, col, w))
                    col += w
            elif n == N_PTILES - 1 and j > N_CHUNKS - N_TAIL_CHUNKS:
                pass  # consumed by the taper above
            else:
                pieces.append((n, base, CHUNK))
    return pieces


def _build():
    nc = bacc.Bacc(
        "TRN2",
        target_bir_lowering=False,
        debug=False,
        num_devices=N_CORES,
        enable_partition_id=False,
    )
    pieces = _pieces()
    assert len(pieces) == N_PIECES
    xs = nc.dram_tensor("xs", [ROWS, HW], mybir.dt.float32, kind="ExternalInput")
    out = nc.dram_tensor(
        "out", [128, N_PIECES], mybir.dt.float32, kind="ExternalOutput"
    )
    xs_ap = xs.ap()
    out_ap = out.ap()

    with tile.TileContext(nc) as tc:
        with (
            tc.tile_pool(name="data", bufs=DATA_BUFS) as data_pool,
            tc.tile_pool(name="small", bufs=1) as small_pool,
        ):
            stats = small_pool.tile([128, N_PIECES], mybir.dt.float32)
            scratch = small_pool.tile([128, CHUNK], mybir.dt.float32)
            for i, (n, col, width) in enumerate(pieces):
                t_in = data_pool.tile([128, width], mybir.dt.float32, tag="data")
                nc.sync.dma_start(
                    out=t_in[:, 0:width],
                    in_=xs_ap[n * 128 : (n + 1) * 128, col : col + width],
                )
                if i % 2 == 1:
                    nc.vector.reduce_sum(
                        out=stats[:, i : i + 1],
                        in_=t_in[:, 0:width],
                        axis=mybir.AxisListType.X,
                    )
                else:
                    nc.scalar.activation(
                        out=scratch[:, 0:width],
                        in_=t_in[:, 0:width],
                        func=mybir.ActivationFunctionType.Copy,
                        accum_out=stats[:, i : i + 1],
                    )
                if i == N_FULL - 1:
                    # bulk of the output leaves while the tail still streams;
                    # SWDGE (GpSimd) so its reduce-waits never block the Sync
                    # data ring
                    nc.gpsimd.dma_start(
                        out=out_ap[:, 0:N_FULL], in_=stats[:, 0:N_FULL]
                    )

            nc.sync.dma_start(
                out=out_ap[:, N_FULL:N_PIECES], in_=stats[:, N_FULL:N_PIECES]
            )

    nc.compile()
    return nc


def _get_nc():
    if "nc" not in _CACHE:
        _CACHE["nc"] = _build()
    return _CACHE["nc"]


def _in_maps(x: np.ndarray):
    x = np.ascontiguousarray(np.asarray(x, dtype=np.float32))
    return [
        {"xs": x[i * B_LOC : (i + 1) * B_LOC].reshape(ROWS, HW)}
        for i in range(N_CORES)
    ]


def _finish(per_core_stats) -> np.ndarray:
    """Fold 8 cores' [128, N_PIECES] partial sums -> (B, C) output."""
    pieces = _pieces()
    total = np.zeros(C, dtype=np.float64)
    for st in per_core_stats:
        st = np.asarray(st, dtype=np.float64)  # [128, N_PIECES]
        row_sums = np.zeros(ROWS, dtype=np.float64)
        for i, (n, _col, _w) in enumerate(pieces):
            row_sums[n * 128 : (n + 1) * 128] += st[:, i]
        total += row_sums.reshape(B_LOC, C).sum(axis=0)
    scores = 1.0 / (1.0 + np.exp(-(total * MEAN_SCALE)))
    return np.broadcast_to(
        scores.astype(np.float32)[None, :], (B, C)
    ).copy()


def _run(x: np.ndarray, **kwargs):
    return run_bass_kernel_spmd(_get_nc(), _in_maps(x), list(range(N_CORES)), **kwargs)


def kernel(x: np.ndarray) -> np.ndarray:
    res = _run(x)
    return _finish([res.results[i]["out"] for i in range(N_CORES)])
